# Initial kernel scaffold
#
"""Trainium2 Bass kernel for nn_DecoderTransformer (B=2, T=2048, C=512, H=8,
L=4, V=50257) on 8 NeuronCores.

Sharding:
 - Rows (B*T = 4096 tokens) are split 512/core for embedding, LayerNorm, FF,
   projection and the residual stream, all kept feature-major [C, rows] so the
   token-dim reductions (LN stats) ride on the PE via an all-ones matmul.
 - Attention is head-sharded: core r owns head r for both batches (uniform
   causal work => perfect SPMD balance). Per layer: AllGather of the LN1
   output (so every core can form q/k/v of its head for all rows) and an
   AllToAll of the attention outputs (head-blocks -> row-blocks).
 - lm_head is vocab-sharded: each core computes logits[:, chunk] plus the
   per-row sum(exp(logit)) over its chunk; the host combines the partials
   into the cross-entropy loss (pure glue: log/add/gather of 4096 scalars).

All matmuls run in float32r (TF32-like, ~1.2e-4 rounding) at full PE rate.
"""

import numpy as np

import concourse.bass as bass
import concourse.mybir as mybir
import concourse.tile as tile
from concourse import bacc
from concourse.bass_utils import run_bass_kernel_spmd
from concourse.masks import make_identity

R = 8          # cores
P = 128        # partitions
B, T = 2, 2048
C, H, HD = 512, 8, 64
CS = C // P    # 4 C-subtiles
FF = 4 * C     # 2048
FS = FF // P   # 16 hidden subtiles
L = 4
V = 50257
NR = B * T     # 4096 rows
RO = NR // R   # 512 own rows per core
RC = RO // P   # 4 own row chunks
VC = 6656      # padded vocab chunk (13 x 512)
VT = VC // 512
EPS = 1e-5
SCALE = 1.0 / 8.0  # 1/sqrt(HD)

F32 = mybir.dt.float32
F32R = mybir.dt.float32r
I32 = mybir.dt.int32
AF = mybir.ActivationFunctionType
ALU = mybir.AluOpType


def build_nc():
    nc = bacc.Bacc("TRN2", target_bir_lowering=False, debug=False, num_devices=R)

    def inp(name, shape, dt=F32):
        return nc.dram_tensor(name, shape, dt, kind="ExternalInput").ap()

    x_idx = inp("x_idx", [P, RC], I32)
    pos_pc = inp("pos_pc", [RO, C])
    tok_emb = inp("tok_emb", [V, C])
    ones_r = inp("ones_r", [P, P], F32R)
    masks = inp("masks", [2, P, 256], F32R)
    ln_pc = inp("ln_pc", [P, L, 2, 2, CS])     # [p, l, ln1/2, scale/bias, sub]
    lnf_pc = inp("lnf_pc", [P, 2, CS])
    bpj_pc = inp("bpj_pc", [P, L, CS])
    bf1_pc = inp("bf1_pc", [P, L, FS])
    bf2_pc = inp("bf2_pc", [P, L, CS])
    wqk_pc = inp("wqk_pc", [L, CS, P, P], F32R)
    wv_pc = inp("wv_pc", [L, CS, P, HD], F32R)
    wpj_pc = inp("wpj_pc", [L, CS, P, C], F32R)
    wf1_pc = inp("wf1_pc", [L, CS, P, FF], F32R)
    wf2_pc = inp("wf2_pc", [L, FS, P, C], F32R)
    lmw_pc = inp("lmw_pc", [CS, P, VC], F32R)

    logits_o = nc.dram_tensor("logits_o", [NR, VC], F32, kind="ExternalOutput").ap()
    sumexp_o = nc.dram_tensor("sumexp_o", [P, NR // P], F32, kind="ExternalOutput").ap()

    rg = [list(range(R))]

    with tile.TileContext(nc) as tc:
        with (
            tc.tile_pool(name="const", bufs=1) as const,
            tc.tile_pool(name="persist", bufs=1) as pers,
            tc.tile_pool(name="wsmall", bufs=2) as wsm,
            tc.tile_pool(name="stat", bufs=2) as statp,
            tc.tile_pool(name="dram", bufs=1, space="DRAM") as dram,
            tc.tile_pool(name="ps_stat", bufs=2, space="PSUM") as ps_stat,
            tc.tile_pool(name="ps_mm", bufs=4, space="PSUM") as ps_mm,
        ):
            # ---- constants ----
            ones_sb = const.tile([P, P], F32R)
            nc.sync.dma_start(ones_sb[:], ones_r[:])
            ident = const.tile([P, P], F32)
            make_identity(nc, ident)
            eps_t = const.tile([P, 1], F32)
            nc.vector.memset(eps_t[:], EPS)
            mask_sb = const.tile([2, P, 256], F32R)
            nc.sync.dma_start(
                mask_sb[:].rearrange("p m n -> (p m) n").rearrange("(m p) n -> p (m n)", p=P),
                masks.rearrange("m p n -> p (m n)"),
            )
            ln_sb = const.tile([P, L, 2, 2, CS], F32)
            nc.sync.dma_start(ln_sb[:], ln_pc[:])
            lnf_sb = const.tile([P, 2, CS], F32)
            nc.sync.dma_start(lnf_sb[:], lnf_pc[:])
            bpj_sb = const.tile([P, L, CS], F32)
            nc.sync.dma_start(bpj_sb[:], bpj_pc[:])
            bf1_sb = const.tile([P, L, FS], F32)
            nc.sync.dma_start(bf1_sb[:], bf1_pc[:])
            bf2_sb = const.tile([P, L, CS], F32)
            nc.sync.dma_start(bf2_sb[:], bf2_pc[:])

            # ---- residual stream (feature-major, own rows) ----
            hT = pers.tile([P, CS, RO], F32R)

            # ---- embedding: gather + pos add, then transpose to f-major ----
            with tc.tile_pool(name="emb", bufs=1) as embp:
                idx_sb = embp.tile([P, RC], I32)
                nc.sync.dma_start(idx_sb[:], x_idx[:])
                emb_tm = embp.tile([P, RC, C], F32)
                for rc in range(RC):
                    nc.gpsimd.indirect_dma_start(
                        out=emb_tm[:, rc, :],
                        out_offset=None,
                        in_=tok_emb[:],
                        in_offset=bass.IndirectOffsetOnAxis(
                            ap=idx_sb[:, rc : rc + 1], axis=0
                        ),
                    )
                pos_sb = embp.tile([P, RC, C], F32)
                nc.sync.dma_start(
                    pos_sb[:], pos_pc.rearrange("(c p) d -> p c d", p=P)
                )
                nc.vector.tensor_tensor(
                    emb_tm[:], emb_tm[:], pos_sb[:], ALU.add
                )
                for rc in range(RC):
                    for s in range(CS):
                        tp = ps_mm.tile([P, P], F32, tag="tr")
                        nc.tensor.transpose(
                            tp[:], emb_tm[:, rc, s * P : (s + 1) * P], ident[:]
                        )
                        nc.vector.tensor_copy(
                            hT[:, s, rc * P : (rc + 1) * P], tp[:]
                        )

            # ---- LayerNorm helper (feature-major, stats via ones-matmul) ----
            def layer_norm(src, dst, sc_ap, bi_ap):
                # src [P, CS, RO] f32r -> dst [P, CS, RO] f32r normalized
                sq = statp.tile([P, CS, RO], F32R, tag="sq")
                nc.scalar.activation(sq[:], src[:], AF.Square)
                pm = ps_stat.tile([P, RO], F32, tag="pmean")
                pq = ps_stat.tile([P, RO], F32, tag="psq")
                for s in range(CS):
                    nc.tensor.matmul(
                        pm[:], lhsT=ones_sb[:], rhs=src[:, s, :],
                        start=(s == 0), stop=(s == CS - 1),
                    )
                for s in range(CS):
                    nc.tensor.matmul(
                        pq[:], lhsT=ones_sb[:], rhs=sq[:, s, :],
                        start=(s == 0), stop=(s == CS - 1),
                    )
                m_sb = statp.tile([P, RO], F32, tag="m")
                nc.vector.tensor_scalar_mul(m_sb[:], pm[:], 1.0 / C)
                ms = statp.tile([P, RO], F32, tag="ms")
                nc.vector.tensor_mul(ms[:], m_sb[:], m_sb[:])
                var = statp.tile([P, RO], F32, tag="var")
                nc.vector.scalar_tensor_tensor(
                    out=var[:], in0=pq[:], scalar=1.0 / C, in1=ms[:],
                    op0=ALU.mult, op1=ALU.subtract,
                )
                std = statp.tile([P, RO], F32, tag="std")
                nc.scalar.activation(std[:], var[:], AF.Sqrt, bias=eps_t[:])
                rstd = statp.tile([P, RO], F32, tag="rstd")
                nc.vector.reciprocal(rstd[:], std[:])
                cen = statp.tile([P, CS, RO], F32, tag="cen")
                nc.vector.tensor_tensor(
                    cen[:], src[:],
                    m_sb[:, None, :].to_broadcast((P, CS, RO)), ALU.subtract,
                )
                nc.vector.tensor_tensor(
                    dst[:], cen[:],
                    rstd[:, None, :].to_broadcast((P, CS, RO)), ALU.mult,
                )
                for s in range(CS):
                    nc.vector.tensor_scalar(
                        out=dst[:, s, :], in0=dst[:, s, :],
                        scalar1=sc_ap[s], scalar2=bi_ap[s],
                        op0=ALU.mult, op1=ALU.add,
                    )

            a_in = pers.tile([P, CS, RO], F32R)
            f_in = pers.tile([P, CS, RO], F32R)

            # ---- transformer layers ----
            for l in range(L):
                layer_norm(
                    hT, a_in,
                    [ln_sb[:, l, 0, 0, s : s + 1] for s in range(CS)],
                    [ln_sb[:, l, 0, 1, s : s + 1] for s in range(CS)],
                )

                # a_in -> DRAM -> AllGather
                ag1_in = dram.tile([CS, P, RO], F32R, tag=f"ag1i_{l}")
                nc.sync.dma_start(
                    ag1_in[:].rearrange("s p t -> p s t"), a_in[:]
                )
                ag1_out = dram.tile(
                    [R, CS, P, RO], F32R, addr_space="Shared", tag=f"ag1o_{l}"
                )
                nc.gpsimd.collective_compute(
                    "AllGather", ALU.bypass, replica_groups=rg,
                    ins=[ag1_in[:].opt()], outs=[ag1_out[:].opt()],
                )

                with (
                    tc.tile_pool(name=f"attn_{l}", bufs=1) as ap_,
                    tc.tile_pool(name=f"attn2_{l}", bufs=4) as ap2,
                    tc.tile_pool(name=f"ps_at_{l}", bufs=4, space="PSUM") as ps_at,
                    tc.tile_pool(name=f"ps_av_{l}", bufs=2, space="PSUM") as ps_av,
                ):
                    # gathered activations [P, CS, NR] (rank blocks = row blocks)
                    a_full = ap_.tile([P, CS, NR], F32R)
                    nc.sync.dma_start(
                        a_full[:].rearrange("p s (b t) -> p s b t", b=R),
                        ag1_out[:].rearrange("b s p t -> p s b t"),
                    )

                    wqk_sb = wsm.tile([P, CS, P], F32R, tag="wqk")
                    nc.sync.dma_start(
                        wqk_sb[:], wqk_pc[l].rearrange("s p m -> p s m")
                    )
                    wv_sb = wsm.tile([P, CS, HD], F32R, tag="wv")
                    nc.sync.dma_start(
                        wv_sb[:], wv_pc[l].rearrange("s p m -> p s m")
                    )
                    wpj_sb = wsm.tile([P, CS, C], F32R, tag="wpj")
                    nc.sync.dma_start(
                        wpj_sb[:], wpj_pc[l].rearrange("s p m -> p s m")
                    )

                    # q,k feature-major [128 = 64 dims x 2 batches, T]
                    qT = ap_.tile([P, T], F32R)
                    kT = ap_.tile([P, T], F32R)
                    for n in range(NR // 512):
                        pqk = ps_mm.tile([P, 512], F32, tag="pqk")
                        for s in range(CS):
                            nc.tensor.matmul(
                                pqk[:], lhsT=wqk_sb[:, s, :],
                                rhs=a_full[:, s, n * 512 : (n + 1) * 512],
                                start=(s == 0), stop=(s == CS - 1),
                            )
                        b, col = n // RC, (n % RC) * 512
                        nc.vector.tensor_copy(
                            qT[64 * b : 64 * b + 64, col : col + 512], pqk[0:64, :]
                        )
                        nc.vector.tensor_copy(
                            kT[64 * b : 64 * b + 64, col : col + 512], pqk[64:128, :]
                        )

                    # v token-major with ones columns [P, 32, 128]
                    v_sb = ap_.tile([P, NR // P, P], F32R)
                    nc.vector.tensor_copy(
                        v_sb[:, :, HD:P],
                        ones_sb[:, None, 0:HD].to_broadcast((P, NR // P, HD)),
                    )
                    for ch in range(NR // P):
                        pv = ps_mm.tile([P, HD], F32, tag="pv")
                        for s in range(CS):
                            nc.tensor.matmul(
                                pv[:], lhsT=a_full[:, s, ch * P : (ch + 1) * P],
                                rhs=wv_sb[:, s, :],
                                start=(s == 0), stop=(s == CS - 1),
                            )
                        nc.vector.tensor_copy(v_sb[:, ch, 0:HD], pv[:])

                    # causal attention, q blocks of 256, both batches in parallel
                    attn_sb = ap_.tile([P, T], F32R)
                    for qb in range(T // 256):
                        pav = [
                            ps_av.tile([P, 256], F32, tag=f"pav{b}")
                            for b in range(B)
                        ]
                        nkv = 2 * qb + 2
                        for j in range(nkv):
                            psc = [
                                ps_at.tile([P, 256], F32, tag=f"psc{b}")
                                for b in range(B)
                            ]
                            for b in range(B):
                                nc.tensor.matmul(
                                    psc[b][:],
                                    lhsT=kT[64 * b : 64 * b + 64, j * P : (j + 1) * P],
                                    rhs=qT[64 * b : 64 * b + 64, qb * 256 : qb * 256 + 256],
                                    start=True, stop=True,
                                    tile_position=(64 * b, 0),
                                )
                            for b in range(B):
                                ex = ap2.tile([P, 256], F32R, tag=f"ex{b}")
                                nc.scalar.activation(
                                    ex[:], psc[b][:], AF.Exp, scale=SCALE
                                )
                                if j >= nkv - 2:
                                    nc.vector.tensor_tensor(
                                        ex[:], ex[:],
                                        mask_sb[j - (nkv - 2), :, :], ALU.mult,
                                    )
                                nc.tensor.matmul(
                                    pav[b][:],
                                    lhsT=v_sb[:, b * (T // P) + j, :],
                                    rhs=ex[:],
                                    start=(j == 0), stop=(j == nkv - 1),
                                )
                        for b in range(B):
                            rec = ap2.tile([HD, 256], F32, tag=f"rec{b}")
                            nc.vector.reciprocal(rec[:], pav[b][64:128, :])
                            nc.vector.tensor_tensor(
                                attn_sb[64 * b : 64 * b + 64, qb * 256 : qb * 256 + 256],
                                pav[b][0:64, :], rec[:], ALU.mult,
                            )

                    # AllToAll: head blocks -> row blocks
                    a2a_in = dram.tile([R, HD, RO], F32R, tag=f"a2i_{l}")
                    for d in range(R):
                        nc.sync.dma_start(
                            a2a_in[d],
                            attn_sb[
                                64 * (d // RC) : 64 * (d // RC) + 64,
                                (d % RC) * RO : (d % RC + 1) * RO,
                            ],
                        )
                    a2a_out = dram.tile(
                        [R, HD, RO], F32R, addr_space="Shared", tag=f"a2o_{l}"
                    )
                    nc.gpsimd.collective_compute(
                        "AllToAll", ALU.bypass, replica_groups=rg,
                        ins=[a2a_in[:].opt()], outs=[a2a_out[:].opt()],
                    )

                    # proj + residual (own rows)
                    af = ap_.tile([P, CS, RO], F32R)
                    for cc in range(CS):
                        nc.sync.dma_start(
                            af[:, cc, :],
                            a2a_out[2 * cc : 2 * cc + 2].rearrange(
                                "h d t -> (h d) t"
                            ),
                        )
                    for cc in range(CS):
                        ppr = ps_mm.tile([P, RO], F32, tag="ppr")
                        for s in range(CS):
                            nc.tensor.matmul(
                                ppr[:], lhsT=wpj_sb[:, s, cc * P : (cc + 1) * P],
                                rhs=af[:, s, :],
                                start=(s == 0), stop=(s == CS - 1),
                            )
                        nc.vector.scalar_tensor_tensor(
                            out=hT[:, cc, :], in0=ppr[:],
                            scalar=bpj_sb[:, l, cc : cc + 1], in1=hT[:, cc, :],
                            op0=ALU.add, op1=ALU.add,
                        )

                layer_norm(
                    hT, f_in,
                    [ln_sb[:, l, 1, 0, s : s + 1] for s in range(CS)],
                    [ln_sb[:, l, 1, 1, s : s + 1] for s in range(CS)],
                )

                # feed-forward
                with (
                    tc.tile_pool(name=f"ff_{l}", bufs=1) as ffp,
                    tc.tile_pool(name=f"ffh_{l}", bufs=3) as ffh,
                    tc.tile_pool(name=f"ps_f1_{l}", bufs=2, space="PSUM") as ps_f1,
                    tc.tile_pool(name=f"ps_f2_{l}", bufs=4, space="PSUM") as ps_f2,
                ):
                    wf1_sb = ffp.tile([P, CS, FF], F32R)
                    nc.sync.dma_start(
                        wf1_sb[:], wf1_pc[l].rearrange("s p m -> p s m")
                    )
                    wf2_sb = ffp.tile([P, FS, C], F32R)
                    nc.sync.dma_start(
                        wf2_sb[:], wf2_pc[l].rearrange("s p m -> p s m")
                    )
                    po = [ps_f2.tile([P, RO], F32, tag=f"po{cc}") for cc in range(CS)]
                    for hc in range(FS):
                        ph = ps_f1.tile([P, RO], F32, tag="ph")
                        for s in range(CS):
                            nc.tensor.matmul(
                                ph[:], lhsT=wf1_sb[:, s, hc * P : (hc + 1) * P],
                                rhs=f_in[:, s, :],
                                start=(s == 0), stop=(s == CS - 1),
                            )
                        hid = ffh.tile([P, RO], F32R, tag="hid")
                        nc.scalar.activation(
                            hid[:], ph[:], AF.Relu, bias=bf1_sb[:, l, hc : hc + 1]
                        )
                        for cc in range(CS):
                            nc.tensor.matmul(
                                po[cc][:], lhsT=wf2_sb[:, hc, cc * P : (cc + 1) * P],
                                rhs=hid[:],
                                start=(hc == 0), stop=(hc == FS - 1),
                            )
                    for cc in range(CS):
                        nc.vector.scalar_tensor_tensor(
                            out=hT[:, cc, :], in0=po[cc][:],
                            scalar=bf2_sb[:, l, cc : cc + 1], in1=hT[:, cc, :],
                            op0=ALU.add, op1=ALU.add,
                        )

            # ---- final LN + AllGather ----
            layer_norm(
                hT, a_in,
                [lnf_sb[:, 0, s : s + 1] for s in range(CS)],
                [lnf_sb[:, 1, s : s + 1] for s in range(CS)],
            )
            ag3_in = dram.tile([CS, P, RO], F32R, tag="ag3i")
            nc.sync.dma_start(ag3_in[:].rearrange("s p t -> p s t"), a_in[:])
            ag3_out = dram.tile([R, CS, P, RO], F32R, addr_space="Shared", tag="ag3o")
            nc.gpsimd.collective_compute(
                "AllGather", ALU.bypass, replica_groups=rg,
                ins=[ag3_in[:].opt()], outs=[ag3_out[:].opt()],
            )

            # ---- lm_head (vocab chunk) + sum-exp partials ----
            with (
                tc.tile_pool(name="lm", bufs=1) as lmp,
                tc.tile_pool(name="lmw", bufs=2) as lmwp,
                tc.tile_pool(name="lms", bufs=2) as lms,
                tc.tile_pool(name="ps_lm", bufs=6, space="PSUM") as ps_lm,
            ):
                lnf_all = lmp.tile([P, CS, NR], F32R)
                nc.sync.dma_start(
                    lnf_all[:].rearrange("p s (b t) -> p s b t", b=R),
                    ag3_out[:].rearrange("b s p t -> p s b t"),
                )
                se_sb = lmp.tile([P, (NR // P) * VT], F32)
                for vt in range(VT):
                    lmw_sb = lmwp.tile([P, CS, 512], F32R, tag="lmw")
                    nc.sync.dma_start(
                        lmw_sb[:],
                        lmw_pc[:, :, vt * 512 : (vt + 1) * 512].rearrange(
                            "s p m -> p s m"
                        ),
                    )
                    for rc in range(NR // P):
                        pl = ps_lm.tile([P, 512], F32, tag="pl")
                        for s in range(CS):
                            nc.tensor.matmul(
                                pl[:], lhsT=lnf_all[:, s, rc * P : (rc + 1) * P],
                                rhs=lmw_sb[:, s, :],
                                start=(s == 0), stop=(s == CS - 1),
                            )
                        lg = lms.tile([P, 512], F32, tag="lg")
                        nc.vector.tensor_copy(lg[:], pl[:])
                        nc.sync.dma_start(
                            logits_o[rc * P : (rc + 1) * P, vt * 512 : (vt + 1) * 512],
                            lg[:],
                        )
                        scr = lms.tile([P, 512], F32, tag="scr")
                        nc.scalar.activation(
                            scr[:], lg[:], AF.Exp,
                            accum_out=se_sb[:, rc * VT + vt : rc * VT + vt + 1],
                        )
                se_tot = lmp.tile([P, NR // P], F32)
                nc.vector.reduce_sum(
                    se_tot[:],
                    se_sb[:].rearrange("p (rc v) -> p rc v", v=VT),
                    axis=mybir.AxisListType.X,
                )
                nc.sync.dma_start(sumexp_o[:], se_tot[:])

    nc.compile()
    return nc


_NC_CACHE = None


def _get_nc():
    global _NC_CACHE
    if _NC_CACHE is None:
        _NC_CACHE = build_nc()
    return _NC_CACHE


def kernel(x, target, tok_emb, pos_emb, ln1_scale, ln1_bias, wq, wk, wv,
           w_proj, b_proj, ln2_scale, ln2_bias, w_ff1, b_ff1, w_ff2, b_ff2,
           lnf_scale, lnf_bias, lm_w, lm_b):
    f32 = np.float32
    x = np.asarray(x, np.int32)
    target = np.asarray(target, np.int32)
    tok_emb = np.ascontiguousarray(np.asarray(tok_emb, f32))
    pos_emb = np.asarray(pos_emb, f32)
    ln_params = np.stack(
        [
            np.stack([np.asarray(ln1_scale, f32), np.asarray(ln1_bias, f32)], 1),
            np.stack([np.asarray(ln2_scale, f32), np.asarray(ln2_bias, f32)], 1),
        ],
        1,
    )  # [L, 2, 2, C]
    wq = np.asarray(wq, f32)
    wk = np.asarray(wk, f32)
    wv = np.asarray(wv, f32)
    w_proj = np.asarray(w_proj, f32)
    b_proj = np.asarray(b_proj, f32)
    w_ff1 = np.asarray(w_ff1, f32)
    b_ff1 = np.asarray(b_ff1, f32)
    w_ff2 = np.asarray(w_ff2, f32)
    b_ff2 = np.asarray(b_ff2, f32)
    lnf = np.stack([np.asarray(lnf_scale, f32), np.asarray(lnf_bias, f32)], 0)
    lm_w = np.asarray(lm_w, f32)
    lm_b = np.asarray(lm_b, f32)
    assert np.all(lm_b == 0.0) and np.all(b_proj == 0.0), \
        "device kernel folds zero biases; nonzero lm_b/b_proj unsupported"

    x_flat = x.reshape(-1)

    # vocab chunking: core r covers [r*6283, min(V, (r+1)*6283)) padded to VC
    CW = 6283
    widths = [min(V, (r + 1) * CW) - r * CW for r in range(R)]

    def perm_cpart(a):
        # [..., C] with C = s*128+p -> [..., CS, P] -> [P, ..., CS]
        sh = a.shape[:-1]
        b = a.reshape(*sh, CS, P)
        return np.ascontiguousarray(np.moveaxis(b, -1, 0))

    ln_pc = perm_cpart(ln_params)                     # [P, L, 2, 2, CS]
    lnf_pc = perm_cpart(lnf)                          # [P, 2, CS]
    bpj_pc = perm_cpart(b_proj)                       # [P, L, CS]
    bf2_pc = perm_cpart(b_ff2)                        # [P, L, CS]
    bf1_pc = np.ascontiguousarray(
        np.moveaxis(b_ff1.reshape(L, FS, P), -1, 0)
    )                                                 # [P, L, FS]

    ones_np = np.ones((P, P), f32)
    kvl = np.arange(P)[:, None]
    ql = np.arange(256)[None, :]
    masks_np = np.stack(
        [(ql >= kvl).astype(f32), (ql >= kvl + 128).astype(f32)], 0
    )

    wpj_np = w_proj.reshape(L, CS, P, C)
    wf1_np = w_ff1.reshape(L, CS, P, FF)
    wf2_np = w_ff2.reshape(L, FS, P, C)

    in_maps = []
    for r in range(R):
        rows = slice(r * RO, (r + 1) * RO)
        x_own = x_flat[rows].reshape(RC, P).T          # [P, RC]
        t0 = (r % RC) * RO
        pos_own = pos_emb[t0 : t0 + RO]
        wqk = np.concatenate([wq[:, r], wk[:, r]], axis=2)   # [L, C, 128]
        lmw = np.zeros((C, VC), f32)
        lmw[:, : widths[r]] = lm_w[:, r * CW : r * CW + widths[r]]
        in_maps.append(
            {
                "x_idx": np.ascontiguousarray(x_own),
                "pos_pc": np.ascontiguousarray(pos_own),
                "tok_emb": tok_emb,
                "ones_r": ones_np,
                "masks": masks_np,
                "ln_pc": ln_pc,
                "lnf_pc": lnf_pc,
                "bpj_pc": bpj_pc,
                "bf1_pc": bf1_pc,
                "bf2_pc": bf2_pc,
                "wqk_pc": np.ascontiguousarray(wqk.reshape(L, CS, P, P)),
                "wv_pc": np.ascontiguousarray(wv[:, r].reshape(L, CS, P, HD)),
                "wpj_pc": np.ascontiguousarray(wpj_np),
                "wf1_pc": np.ascontiguousarray(wf1_np),
                "wf2_pc": np.ascontiguousarray(wf2_np),
                "lmw_pc": np.ascontiguousarray(lmw.reshape(CS, P, VC)),
            }
        )

    nc = _get_nc()
    res = run_bass_kernel_spmd(nc, in_maps, core_ids=list(range(R)))
    outs = res.results

    logits = np.concatenate(
        [outs[r]["logits_o"][:, : widths[r]] for r in range(R)], axis=1
    )

    # CE loss from device partials (host glue on 4096-vectors)
    npad = np.array([VC - widths[r] for r in range(R)], f32)
    se = np.stack(
        [outs[r]["sumexp_o"].T.reshape(-1) - npad[r] for r in range(R)], 0
    )  # [R, NR]
    logz = np.log(se.sum(0))
    tgt = logits[np.arange(NR), target.reshape(-1)]
    loss = np.float32(-(tgt - logz).mean())

    return logits, loss


# revision 8
# speedup vs baseline: 1.0412x; 1.0412x over previous
"""Trainium2 Bass kernel for nn_DecoderTransformer (B=2, T=2048, C=512, H=8,
L=4, V=50257) on 8 NeuronCores.

Sharding:
 - Rows (B*T = 4096 tokens) are split 512/core for embedding, LayerNorm, FF,
   projection and the residual stream, all kept feature-major [C, rows] so the
   token-dim reductions (LN stats) ride on the PE via an all-ones matmul.
 - Attention is head-sharded: core r owns head r for both batches (uniform
   causal work => perfect SPMD balance). Per layer: AllGather of the LN1
   output (so every core can form q/k/v of its head for all rows) and an
   AllToAll of the attention outputs (head-blocks -> row-blocks).
 - lm_head is vocab-sharded: each core computes logits[:, chunk] plus the
   per-row sum(exp(logit)) over its chunk; the host combines the partials
   into the cross-entropy loss (pure glue: log/add/gather of 4096 scalars).

All matmuls run in float32r (TF32-like, ~1.2e-4 rounding) at full PE rate.
"""

import numpy as np

import concourse.bass as bass
import concourse.mybir as mybir
import concourse.tile as tile
from concourse import bacc
from concourse.bass_utils import run_bass_kernel_spmd
from concourse.masks import make_identity

R = 8          # cores
P = 128        # partitions
B, T = 2, 2048
C, H, HD = 512, 8, 64
CS = C // P    # 4 C-subtiles
FF = 4 * C     # 2048
FS = FF // P   # 16 hidden subtiles
L = 4
V = 50257
NR = B * T     # 4096 rows
RO = NR // R   # 512 own rows per core
RC = RO // P   # 4 own row chunks
VC = 6656      # padded vocab chunk (13 x 512)
VT = VC // 512
EPS = 1e-5
SCALE = 1.0 / 8.0  # 1/sqrt(HD)

F32 = mybir.dt.float32
F32R = mybir.dt.float32r
I32 = mybir.dt.int32
AF = mybir.ActivationFunctionType
ALU = mybir.AluOpType


def build_nc():
    nc = bacc.Bacc("TRN2", target_bir_lowering=False, debug=False, num_devices=R)

    def inp(name, shape, dt=F32):
        return nc.dram_tensor(name, shape, dt, kind="ExternalInput").ap()

    x_idx = inp("x_idx", [P, RC], I32)
    pos_pc = inp("pos_pc", [RO, C])
    tok_emb = inp("tok_emb", [V, C])
    ones_r = inp("ones_r", [P, P], F32R)
    masks = inp("masks", [2, P, 256], F32R)
    ln_pc = inp("ln_pc", [P, L, 2, 2, CS])     # [p, l, ln1/2, scale/bias, sub]
    lnf_pc = inp("lnf_pc", [P, 2, CS])
    bpj_pc = inp("bpj_pc", [P, L, CS])
    bf1_pc = inp("bf1_pc", [P, L, FS])
    bf2_pc = inp("bf2_pc", [P, L, CS])
    wqk_pc = inp("wqk_pc", [L, CS, P, P], F32R)
    wv_pc = inp("wv_pc", [L, CS, P, HD], F32R)
    wpj_pc = inp("wpj_pc", [L, CS, P, C], F32R)
    wf1_pc = inp("wf1_pc", [L, CS, P, FF], F32R)
    wf2_pc = inp("wf2_pc", [L, FS, P, C], F32R)
    lmw_pc = inp("lmw_pc", [CS, P, VC], F32R)

    logits_o = nc.dram_tensor("logits_o", [NR, VC], F32, kind="ExternalOutput").ap()
    sumexp_o = nc.dram_tensor("sumexp_o", [P, NR // P], F32, kind="ExternalOutput").ap()

    rg = [list(range(R))]

    with tile.TileContext(nc) as tc:
        with (
            tc.tile_pool(name="const", bufs=1) as const,
            tc.tile_pool(name="persist", bufs=1) as pers,
            tc.tile_pool(name="wsmall", bufs=2) as wsm,
            tc.tile_pool(name="stat", bufs=1) as statp,
            tc.tile_pool(name="dram", bufs=1, space="DRAM") as dram,
            tc.tile_pool(name="ps", bufs=4, space="PSUM") as ps,
        ):
            # ---- constants ----
            ones_sb = const.tile([P, P], F32R)
            nc.sync.dma_start(ones_sb[:], ones_r[:])
            ident = const.tile([P, P], F32)
            make_identity(nc, ident)
            eps_t = const.tile([P, 1], F32)
            nc.vector.memset(eps_t[:], EPS)
            mask_sb = const.tile([P, 2, 256], F32R)
            nc.sync.dma_start(mask_sb[:], masks.rearrange("m p n -> p m n"))
            ln_sb = const.tile([P, L, 2, 2, CS], F32)
            nc.sync.dma_start(ln_sb[:], ln_pc[:])
            lnf_sb = const.tile([P, 2, CS], F32)
            nc.sync.dma_start(lnf_sb[:], lnf_pc[:])
            bpj_sb = const.tile([P, L, CS], F32)
            nc.sync.dma_start(bpj_sb[:], bpj_pc[:])
            bf1_sb = const.tile([P, L, FS], F32)
            nc.sync.dma_start(bf1_sb[:], bf1_pc[:])
            bf2_sb = const.tile([P, L, CS], F32)
            nc.sync.dma_start(bf2_sb[:], bf2_pc[:])

            # ---- residual stream (feature-major, own rows) ----
            hT = pers.tile([P, CS, RO], F32R)

            # ---- embedding: gather + pos add, then transpose to f-major ----
            with tc.tile_pool(name="emb", bufs=1) as embp:
                idx_sb = embp.tile([P, RC], I32)
                nc.sync.dma_start(idx_sb[:], x_idx[:])
                emb_tm = embp.tile([P, RC, C], F32)
                for rc in range(RC):
                    nc.gpsimd.indirect_dma_start(
                        out=emb_tm[:, rc, :],
                        out_offset=None,
                        in_=tok_emb[:],
                        in_offset=bass.IndirectOffsetOnAxis(
                            ap=idx_sb[:, rc : rc + 1], axis=0
                        ),
                    )
                pos_sb = embp.tile([P, RC, C], F32)
                nc.sync.dma_start(
                    pos_sb[:], pos_pc.rearrange("(c p) d -> p c d", p=P)
                )
                nc.vector.tensor_tensor(
                    emb_tm[:], emb_tm[:], pos_sb[:], ALU.add
                )
                for rc in range(RC):
                    for s in range(CS):
                        tp = ps.tile([P, P], F32, tag="mm")
                        nc.tensor.transpose(
                            tp[:], emb_tm[:, rc, s * P : (s + 1) * P], ident[:]
                        )
                        nc.vector.tensor_copy(
                            hT[:, s, rc * P : (rc + 1) * P], tp[:]
                        )

            # ---- LayerNorm helper (feature-major, stats via ones-matmul) ----
            def layer_norm(src, dst, sc_ap, bi_ap):
                # src [P, CS, RO] f32r -> dst [P, CS, RO] f32r normalized
                sq = statp.tile([P, CS, RO], F32R, tag="sq")
                nc.scalar.activation(sq[:], src[:], AF.Square)
                pm = ps.tile([P, RO], F32, tag="mm")
                pq = ps.tile([P, RO], F32, tag="mm")
                for s in range(CS):
                    nc.tensor.matmul(
                        pm[:], lhsT=ones_sb[:], rhs=src[:, s, :],
                        start=(s == 0), stop=(s == CS - 1),
                    )
                for s in range(CS):
                    nc.tensor.matmul(
                        pq[:], lhsT=ones_sb[:], rhs=sq[:, s, :],
                        start=(s == 0), stop=(s == CS - 1),
                    )
                m_sb = statp.tile([P, RO], F32, tag="m")
                nc.vector.tensor_scalar_mul(m_sb[:], pm[:], 1.0 / C)
                ms = statp.tile([P, RO], F32, tag="ms")
                nc.vector.tensor_mul(ms[:], m_sb[:], m_sb[:])
                var = statp.tile([P, RO], F32, tag="var")
                nc.vector.scalar_tensor_tensor(
                    out=var[:], in0=pq[:], scalar=1.0 / C, in1=ms[:],
                    op0=ALU.mult, op1=ALU.subtract,
                )
                std = statp.tile([P, RO], F32, tag="std")
                nc.scalar.activation(std[:], var[:], AF.Sqrt, bias=eps_t[:])
                rstd = statp.tile([P, RO], F32, tag="rstd")
                nc.vector.reciprocal(rstd[:], std[:])
                cen = statp.tile([P, CS, RO], F32, tag="sq", name="cen")
                nc.vector.tensor_tensor(
                    cen[:], src[:],
                    m_sb[:, None, :].to_broadcast((P, CS, RO)), ALU.subtract,
                )
                nc.vector.tensor_tensor(
                    dst[:], cen[:],
                    rstd[:, None, :].to_broadcast((P, CS, RO)), ALU.mult,
                )
                for s in range(CS):
                    nc.vector.tensor_scalar(
                        out=dst[:, s, :], in0=dst[:, s, :],
                        scalar1=sc_ap[s], scalar2=bi_ap[s],
                        op0=ALU.mult, op1=ALU.add,
                    )

            a_in = pers.tile([P, CS, RO], F32R)
            f_in = pers.tile([P, CS, RO], F32R)

            # ---- transformer layers ----
            for l in range(L):
                layer_norm(
                    hT, a_in,
                    [ln_sb[:, l, 0, 0, s : s + 1] for s in range(CS)],
                    [ln_sb[:, l, 0, 1, s : s + 1] for s in range(CS)],
                )

                # a_in -> DRAM -> AllGather
                ag1_in = dram.tile([CS, P, RO], F32R, tag=f"ag1i_{l}")
                nc.sync.dma_start(
                    ag1_in[:].rearrange("s p t -> p s t"), a_in[:]
                )
                ag1_out = dram.tile(
                    [R, CS, P, RO], F32R, addr_space="Shared", tag=f"ag1o_{l}"
                )
                nc.gpsimd.collective_compute(
                    "AllGather", ALU.bypass, replica_groups=rg,
                    ins=[ag1_in[:].opt()], outs=[ag1_out[:].opt()],
                )

                with (
                    tc.tile_pool(name=f"attn_{l}", bufs=1) as ap_,
                    tc.tile_pool(name=f"attn2_{l}", bufs=4) as ap2,
                    tc.tile_pool(name=f"ps_at_{l}", bufs=2, space="PSUM") as ps_at,
                    tc.tile_pool(name=f"ps_av_{l}", bufs=2, space="PSUM") as ps_av,
                ):

                    wqk_sb = wsm.tile([P, CS, P], F32R, tag="wqk")
                    nc.sync.dma_start(
                        wqk_sb[:], wqk_pc[l].rearrange("s p m -> p s m")
                    )
                    wv_sb = wsm.tile([P, CS, HD], F32R, tag="wv")
                    nc.sync.dma_start(
                        wv_sb[:], wv_pc[l].rearrange("s p m -> p s m")
                    )
                    wpj_sb = wsm.tile([P, CS, C], F32R, tag="wpj")
                    nc.sync.dma_start(
                        wpj_sb[:], wpj_pc[l].rearrange("s p m -> p s m")
                    )

                    # q,k feature-major [128 = 64 dims x 2 batches, T]
                    qT = ap_.tile([P, T], F32R)
                    kT = ap_.tile([P, T], F32R)
                    v_sb = ap_.tile([P, NR // P, P], F32R)
                    nc.vector.tensor_copy(
                        v_sb[:, :, HD:P],
                        ones_sb[:, None, 0:HD].to_broadcast((P, NR // P, HD)),
                    )
                    for half in range(2):  # half == batch (rows half*2048..)
                        a_half = ap_.tile(
                            [P, CS, NR // 2], F32R, tag="a_half",
                            name=f"ah_{l}_{half}",
                        )
                        for b in range(RC):
                            nc.sync.dma_start(
                                a_half[:, :, b * RO : (b + 1) * RO],
                                ag1_out[RC * half + b].rearrange("s p t -> p s t"),
                            )
                        for n4 in range(RC):
                            pqk = ps.tile([P, 512], F32, tag="mm", name=f"pqk{half}_{n4}")
                            for s in range(CS):
                                nc.tensor.matmul(
                                    pqk[:], lhsT=wqk_sb[:, s, :],
                                    rhs=a_half[:, s, n4 * 512 : (n4 + 1) * 512],
                                    start=(s == 0), stop=(s == CS - 1),
                                )
                            col = n4 * 512
                            nc.vector.tensor_copy(
                                qT[64 * half : 64 * half + 64, col : col + 512],
                                pqk[0:64, :],
                            )
                            nc.vector.tensor_copy(
                                kT[64 * half : 64 * half + 64, col : col + 512],
                                pqk[64:128, :],
                            )
                        for ch in range(T // P):
                            pv = ps.tile([P, HD], F32, tag="mm", name=f"pv{half}_{ch}")
                            for s in range(CS):
                                nc.tensor.matmul(
                                    pv[:], lhsT=a_half[:, s, ch * P : (ch + 1) * P],
                                    rhs=wv_sb[:, s, :],
                                    start=(s == 0), stop=(s == CS - 1),
                                )
                            nc.vector.tensor_copy(
                                v_sb[:, half * (T // P) + ch, 0:HD], pv[:]
                            )

                    # causal attention, q blocks of 256, both batches in parallel
                    attn_sb = ap_.tile([P, T], F32R)
                    for qb in range(T // 256):
                        pav = [
                            ps_av.tile([P, 256], F32, tag="pav", name=f"pav{qb}_{b}")
                            for b in range(B)
                        ]
                        nkv = 2 * qb + 2
                        for j in range(nkv):
                            psc = [
                                ps_at.tile([P, 256], F32, tag="psc", name=f"psc{qb}_{j}_{b}")
                                for b in range(B)
                            ]
                            for b in range(B):
                                nc.tensor.matmul(
                                    psc[b][:],
                                    lhsT=kT[64 * b : 64 * b + 64, j * P : (j + 1) * P],
                                    rhs=qT[64 * b : 64 * b + 64, qb * 256 : qb * 256 + 256],
                                    start=True, stop=True,
                                    tile_position=(64 * b, 0),
                                )
                            for b in range(B):
                                ex = ap2.tile([P, 256], F32R, tag=f"ex{b}")
                                nc.scalar.activation(
                                    ex[:], psc[b][:], AF.Exp, scale=SCALE
                                )
                                if j >= nkv - 2:
                                    nc.vector.tensor_tensor(
                                        ex[:], ex[:],
                                        mask_sb[:, j - (nkv - 2), :], ALU.mult,
                                    )
                                nc.tensor.matmul(
                                    pav[b][:],
                                    lhsT=v_sb[:, b * (T // P) + j, :],
                                    rhs=ex[:],
                                    start=(j == 0), stop=(j == nkv - 1),
                                )
                        for b in range(B):
                            rec = ap2.tile([HD, 256], F32, tag=f"rec{b}")
                            nc.vector.reciprocal(rec[:], pav[b][64:128, :])
                            nc.vector.tensor_tensor(
                                attn_sb[64 * b : 64 * b + 64, qb * 256 : qb * 256 + 256],
                                pav[b][0:64, :], rec[:], ALU.mult,
                            )

                    # AllToAll: head blocks -> row blocks
                    a2a_in = dram.tile([R, HD, RO], F32R, tag=f"a2i_{l}")
                    for d in range(R):
                        nc.sync.dma_start(
                            a2a_in[d],
                            attn_sb[
                                64 * (d // RC) : 64 * (d // RC) + 64,
                                (d % RC) * RO : (d % RC + 1) * RO,
                            ],
                        )
                    a2a_out = dram.tile([R, HD, RO], F32R, tag=f"a2o_{l}")
                    nc.gpsimd.collective_compute(
                        "AllToAll", ALU.bypass, replica_groups=rg,
                        ins=[a2a_in[:].opt()], outs=[a2a_out[:].opt()],
                    )

                    # proj + residual (own rows)
                    af = ap_.tile([P, CS, RO], F32R)
                    for cc in range(CS):
                        nc.sync.dma_start(
                            af[:, cc, :],
                            a2a_out[2 * cc : 2 * cc + 2].rearrange(
                                "h d t -> (h d) t"
                            ),
                        )
                    for cc in range(CS):
                        ppr = ps.tile([P, RO], F32, tag="mm")
                        for s in range(CS):
                            nc.tensor.matmul(
                                ppr[:], lhsT=wpj_sb[:, s, cc * P : (cc + 1) * P],
                                rhs=af[:, s, :],
                                start=(s == 0), stop=(s == CS - 1),
                            )
                        nc.vector.scalar_tensor_tensor(
                            out=hT[:, cc, :], in0=ppr[:],
                            scalar=bpj_sb[:, l, cc : cc + 1], in1=hT[:, cc, :],
                            op0=ALU.add, op1=ALU.add,
                        )

                layer_norm(
                    hT, f_in,
                    [ln_sb[:, l, 1, 0, s : s + 1] for s in range(CS)],
                    [ln_sb[:, l, 1, 1, s : s + 1] for s in range(CS)],
                )

                # feed-forward
                with tc.tile_pool(name=f"ff_{l}", bufs=1) as ffp:
                    wf1_sb = ffp.tile([P, CS, FF], F32R)
                    nc.sync.dma_start(
                        wf1_sb[:], wf1_pc[l].rearrange("s p m -> p s m")
                    )
                    wf2_sb = ffp.tile([P, FS, C], F32R)
                    nc.sync.dma_start(
                        wf2_sb[:], wf2_pc[l].rearrange("s p m -> p s m")
                    )
                    hid_all = ffp.tile([P, FS, RO], F32R)
                    for hc in range(FS):
                        ph = ps.tile([P, RO], F32, tag="mm")
                        for s in range(CS):
                            nc.tensor.matmul(
                                ph[:], lhsT=wf1_sb[:, s, hc * P : (hc + 1) * P],
                                rhs=f_in[:, s, :],
                                start=(s == 0), stop=(s == CS - 1),
                            )
                        nc.scalar.activation(
                            hid_all[:, hc, :], ph[:], AF.Relu,
                            bias=bf1_sb[:, l, hc : hc + 1],
                        )
                    for cc in range(CS):
                        po = ps.tile([P, RO], F32, tag="mm")
                        for hc in range(FS):
                            nc.tensor.matmul(
                                po[:], lhsT=wf2_sb[:, hc, cc * P : (cc + 1) * P],
                                rhs=hid_all[:, hc, :],
                                start=(hc == 0), stop=(hc == FS - 1),
                            )
                        nc.vector.scalar_tensor_tensor(
                            out=hT[:, cc, :], in0=po[:],
                            scalar=bf2_sb[:, l, cc : cc + 1], in1=hT[:, cc, :],
                            op0=ALU.add, op1=ALU.add,
                        )

            # ---- final LN + AllGather ----
            layer_norm(
                hT, a_in,
                [lnf_sb[:, 0, s : s + 1] for s in range(CS)],
                [lnf_sb[:, 1, s : s + 1] for s in range(CS)],
            )
            ag3_in = dram.tile([CS, P, RO], F32R, tag="ag3i")
            nc.sync.dma_start(ag3_in[:].rearrange("s p t -> p s t"), a_in[:])
            ag3_out = dram.tile([R, CS, P, RO], F32R, addr_space="Shared", tag="ag3o")
            nc.gpsimd.collective_compute(
                "AllGather", ALU.bypass, replica_groups=rg,
                ins=[ag3_in[:].opt()], outs=[ag3_out[:].opt()],
            )

            # ---- lm_head (vocab chunk) + sum-exp partials ----
            with (
                tc.tile_pool(name="lm", bufs=1) as lmp,
                tc.tile_pool(name="lmw", bufs=2) as lmwp,
                tc.tile_pool(name="lms", bufs=3) as lms,
            ):
                lnf_all = lmp.tile([P, CS, NR], F32R)
                for b in range(R):
                    nc.sync.dma_start(
                        lnf_all[:, :, b * RO : (b + 1) * RO],
                        ag3_out[b].rearrange("s p t -> p s t"),
                    )
                se_sb = lmp.tile([P, (NR // P) * VT], F32)
                for vt in range(VT):
                    lmw_sb = lmwp.tile([P, CS, 512], F32R, tag="lmw")
                    nc.sync.dma_start(
                        lmw_sb[:],
                        lmw_pc[:, :, vt * 512 : (vt + 1) * 512].rearrange(
                            "s p m -> p s m"
                        ),
                    )
                    for rc in range(NR // P):
                        pl = ps.tile([P, 512], F32, tag="mm")
                        for s in range(CS):
                            nc.tensor.matmul(
                                pl[:], lhsT=lnf_all[:, s, rc * P : (rc + 1) * P],
                                rhs=lmw_sb[:, s, :],
                                start=(s == 0), stop=(s == CS - 1),
                            )
                        lg = lms.tile([P, 512], F32, tag="lg")
                        nc.vector.tensor_copy(lg[:], pl[:])
                        nc.sync.dma_start(
                            logits_o[rc * P : (rc + 1) * P, vt * 512 : (vt + 1) * 512],
                            lg[:],
                        )
                        scr = lms.tile([P, 512], F32, tag="scr")
                        nc.scalar.activation(
                            scr[:], lg[:], AF.Exp,
                            accum_out=se_sb[:, rc * VT + vt : rc * VT + vt + 1],
                        )
                se_tot = lmp.tile([P, NR // P], F32)
                nc.vector.reduce_sum(
                    se_tot[:],
                    se_sb[:].rearrange("p (rc v) -> p rc v", v=VT),
                    axis=mybir.AxisListType.X,
                )
                nc.sync.dma_start(sumexp_o[:], se_tot[:])

    nc.compile()
    return nc


_NC_CACHE = None
LAST_RESULT = None


def _get_nc():
    global _NC_CACHE
    if _NC_CACHE is None:
        _NC_CACHE = build_nc()
    return _NC_CACHE


def prepare_in_maps(x, target, tok_emb, pos_emb, ln1_scale, ln1_bias, wq, wk,
                    wv, w_proj, b_proj, ln2_scale, ln2_bias, w_ff1, b_ff1,
                    w_ff2, b_ff2, lnf_scale, lnf_bias, lm_w, lm_b):
    f32 = np.float32
    x = np.asarray(x, np.int32)
    target = np.asarray(target, np.int32)
    tok_emb = np.ascontiguousarray(np.asarray(tok_emb, f32))
    pos_emb = np.asarray(pos_emb, f32)
    ln_params = np.stack(
        [
            np.stack([np.asarray(ln1_scale, f32), np.asarray(ln1_bias, f32)], 1),
            np.stack([np.asarray(ln2_scale, f32), np.asarray(ln2_bias, f32)], 1),
        ],
        1,
    )  # [L, 2, 2, C]
    wq = np.asarray(wq, f32)
    wk = np.asarray(wk, f32)
    wv = np.asarray(wv, f32)
    w_proj = np.asarray(w_proj, f32)
    b_proj = np.asarray(b_proj, f32)
    w_ff1 = np.asarray(w_ff1, f32)
    b_ff1 = np.asarray(b_ff1, f32)
    w_ff2 = np.asarray(w_ff2, f32)
    b_ff2 = np.asarray(b_ff2, f32)
    lnf = np.stack([np.asarray(lnf_scale, f32), np.asarray(lnf_bias, f32)], 0)
    lm_w = np.asarray(lm_w, f32)
    lm_b = np.asarray(lm_b, f32)
    assert np.all(lm_b == 0.0) and np.all(b_proj == 0.0), \
        "device kernel folds zero biases; nonzero lm_b/b_proj unsupported"

    x_flat = x.reshape(-1)

    # vocab chunking: core r covers [r*6283, min(V, (r+1)*6283)) padded to VC
    CW = 6283
    widths = [min(V, (r + 1) * CW) - r * CW for r in range(R)]

    def perm_cpart(a):
        # [..., C] with C = s*128+p -> [..., CS, P] -> [P, ..., CS]
        sh = a.shape[:-1]
        b = a.reshape(*sh, CS, P)
        return np.ascontiguousarray(np.moveaxis(b, -1, 0))

    ln_pc = perm_cpart(ln_params)                     # [P, L, 2, 2, CS]
    lnf_pc = perm_cpart(lnf)                          # [P, 2, CS]
    bpj_pc = perm_cpart(b_proj)                       # [P, L, CS]
    bf2_pc = perm_cpart(b_ff2)                        # [P, L, CS]
    bf1_pc = np.ascontiguousarray(
        np.moveaxis(b_ff1.reshape(L, FS, P), -1, 0)
    )                                                 # [P, L, FS]

    ones_np = np.ones((P, P), f32)
    kvl = np.arange(P)[:, None]
    ql = np.arange(256)[None, :]
    masks_np = np.stack(
        [(ql >= kvl).astype(f32), (ql >= kvl + 128).astype(f32)], 0
    )

    wpj_np = w_proj.reshape(L, CS, P, C)
    wf1_np = w_ff1.reshape(L, CS, P, FF)
    wf2_np = w_ff2.reshape(L, FS, P, C)

    in_maps = []
    for r in range(R):
        rows = slice(r * RO, (r + 1) * RO)
        x_own = x_flat[rows].reshape(RC, P).T          # [P, RC]
        t0 = (r % RC) * RO
        pos_own = pos_emb[t0 : t0 + RO]
        wqk = np.concatenate([wq[:, r], wk[:, r]], axis=2)   # [L, C, 128]
        lmw = np.zeros((C, VC), f32)
        lmw[:, : widths[r]] = lm_w[:, r * CW : r * CW + widths[r]]
        in_maps.append(
            {
                "x_idx": np.ascontiguousarray(x_own),
                "pos_pc": np.ascontiguousarray(pos_own),
                "tok_emb": tok_emb,
                "ones_r": ones_np,
                "masks": masks_np,
                "ln_pc": ln_pc,
                "lnf_pc": lnf_pc,
                "bpj_pc": bpj_pc,
                "bf1_pc": bf1_pc,
                "bf2_pc": bf2_pc,
                "wqk_pc": np.ascontiguousarray(wqk.reshape(L, CS, P, P)),
                "wv_pc": np.ascontiguousarray(wv[:, r].reshape(L, CS, P, HD)),
                "wpj_pc": np.ascontiguousarray(wpj_np),
                "wf1_pc": np.ascontiguousarray(wf1_np),
                "wf2_pc": np.ascontiguousarray(wf2_np),
                "lmw_pc": np.ascontiguousarray(lmw.reshape(CS, P, VC)),
            }
        )

    return in_maps, widths


def kernel(**inputs):
    target = np.asarray(inputs["target"], np.int32)
    in_maps, widths = prepare_in_maps(**inputs)
    nc = _get_nc()
    import os as _os
    _tr = bool(_os.environ.get("KERNEL_TRACE"))
    res = run_bass_kernel_spmd(
        nc, in_maps, core_ids=list(range(R)),
        trace=_tr, trace_cores=[0] if _tr else None,
    )
    global LAST_RESULT
    LAST_RESULT = res
    outs = res.results

    logits = np.concatenate(
        [outs[r]["logits_o"][:, : widths[r]] for r in range(R)], axis=1
    )

    # CE loss from device partials (host glue on 4096-vectors)
    npad = np.array([VC - widths[r] for r in range(R)], f32)
    se = np.stack(
        [outs[r]["sumexp_o"].T.reshape(-1) - npad[r] for r in range(R)], 0
    )  # [R, NR]
    logz = np.log(se.sum(0))
    tgt = logits[np.arange(NR), target.reshape(-1)]
    loss = np.float32(-(tgt - logz).mean())

    return logits, loss


# revision 10
# speedup vs baseline: 1.1031x; 1.0595x over previous
"""Trainium2 Bass kernel for nn_DecoderTransformer (B=2, T=2048, C=512, H=8,
L=4, V=50257) on 8 NeuronCores.

Sharding:
 - Rows (B*T = 4096 tokens) are split 512/core for embedding, LayerNorm, FF,
   projection and the residual stream, all kept feature-major [C, rows] so the
   token-dim reductions (LN stats) ride on the PE via an all-ones matmul.
 - Attention is head-sharded: core r owns head r for both batches (uniform
   causal work => perfect SPMD balance). Per layer: AllGather of the LN1
   output (so every core can form q/k/v of its head for all rows) and an
   AllToAll of the attention outputs (head-blocks -> row-blocks).
 - lm_head is vocab-sharded: each core computes logits[:, chunk] plus the
   per-row sum(exp(logit)) over its chunk; the host combines the partials
   into the cross-entropy loss (pure glue: log/add/gather of 4096 scalars).

All matmuls run in float32r (TF32-like, ~1.2e-4 rounding) at full PE rate.
"""

import numpy as np

import concourse.bass as bass
import concourse.mybir as mybir
import concourse.tile as tile
from concourse import bacc
from concourse.bass_utils import run_bass_kernel_spmd
from concourse.masks import make_identity

R = 8          # cores
P = 128        # partitions
B, T = 2, 2048
C, H, HD = 512, 8, 64
CS = C // P    # 4 C-subtiles
FF = 4 * C     # 2048
FS = FF // P   # 16 hidden subtiles
L = 4
V = 50257
NR = B * T     # 4096 rows
RO = NR // R   # 512 own rows per core
RC = RO // P   # 4 own row chunks
VC = 6656      # padded vocab chunk (13 x 512)
VT = VC // 512
EPS = 1e-5
SCALE = 1.0 / 8.0  # 1/sqrt(HD)

F32 = mybir.dt.float32
F32R = mybir.dt.float32r
I32 = mybir.dt.int32
AF = mybir.ActivationFunctionType
ALU = mybir.AluOpType


def build_nc():
    import os as _os
    _nocoll = _os.environ.get("KVAR") == "nocoll"
    nc = bacc.Bacc("TRN2", target_bir_lowering=False, debug=False, num_devices=R)

    def inp(name, shape, dt=F32):
        return nc.dram_tensor(name, shape, dt, kind="ExternalInput").ap()

    x_idx = inp("x_idx", [P, RC], I32)
    pos_pc = inp("pos_pc", [RO, C])
    tok_emb = inp("tok_emb", [V, C])
    ones_r = inp("ones_r", [P, P], F32R)
    masks = inp("masks", [2, P, 256], F32R)
    ln_pc = inp("ln_pc", [P, L, 2, 2, CS])     # [p, l, ln1/2, scale/bias, sub]
    lnf_pc = inp("lnf_pc", [P, 2, CS])
    bpj_pc = inp("bpj_pc", [P, L, CS])
    bf1_pc = inp("bf1_pc", [P, L, FS])
    bf2_pc = inp("bf2_pc", [P, L, CS])
    wqk_pc = inp("wqk_pc", [L, CS, P, P], F32R)
    wv_pc = inp("wv_pc", [L, CS, P, HD], F32R)
    wpj_pc = inp("wpj_pc", [L, CS, P, C], F32R)
    wf1_pc = inp("wf1_pc", [L, CS, P, FF], F32R)
    wf2_pc = inp("wf2_pc", [L, FS, P, C], F32R)
    lmw_pc = inp("lmw_pc", [CS, P, VC], F32R)

    logits_o = nc.dram_tensor("logits_o", [NR, VC], F32, kind="ExternalOutput").ap()
    sumexp_o = nc.dram_tensor("sumexp_o", [P, NR // P], F32, kind="ExternalOutput").ap()

    rg = [list(range(R))]

    with tile.TileContext(nc) as tc:
        def coll(kind, in_ap, out_ap):
            if _nocoll:
                if kind == "AllGather":
                    for _r in range(R):
                        nc.sync.dma_start(out_ap[_r], in_ap)
                else:
                    nc.sync.dma_start(out_ap, in_ap)
                return
            nc.gpsimd.collective_compute(
                kind, ALU.bypass, replica_groups=rg,
                ins=[in_ap.opt()], outs=[out_ap.opt()],
            )

        with (
            tc.tile_pool(name="const", bufs=1) as const,
            tc.tile_pool(name="persist", bufs=1) as pers,
            tc.tile_pool(name="wsmall", bufs=2) as wsm,
            tc.tile_pool(name="stat", bufs=1) as statp,
            tc.tile_pool(name="dram", bufs=1, space="DRAM") as dram,
            tc.tile_pool(name="ps", bufs=4, space="PSUM") as ps,
        ):
            # ---- constants ----
            ones_sb = const.tile([P, P], F32R)
            nc.sync.dma_start(ones_sb[:], ones_r[:])
            ident = const.tile([P, P], F32)
            make_identity(nc, ident)
            eps_t = const.tile([P, 1], F32)
            nc.vector.memset(eps_t[:], EPS)
            mask_sb = const.tile([P, 2, 256], F32R)
            nc.sync.dma_start(mask_sb[:], masks.rearrange("m p n -> p m n"))
            ln_sb = const.tile([P, L, 2, 2, CS], F32)
            nc.sync.dma_start(ln_sb[:], ln_pc[:])
            lnf_sb = const.tile([P, 2, CS], F32)
            nc.sync.dma_start(lnf_sb[:], lnf_pc[:])
            bpj_sb = const.tile([P, L, CS], F32)
            nc.sync.dma_start(bpj_sb[:], bpj_pc[:])
            bf1_sb = const.tile([P, L, FS], F32)
            nc.sync.dma_start(bf1_sb[:], bf1_pc[:])
            bf2_sb = const.tile([P, L, CS], F32)
            nc.sync.dma_start(bf2_sb[:], bf2_pc[:])

            # ---- residual stream (feature-major, own rows) ----
            hT = pers.tile([P, CS, RO], F32R)

            # ---- embedding: gather + pos add, then transpose to f-major ----
            with tc.tile_pool(name="emb", bufs=1) as embp:
                idx_sb = embp.tile([P, RC], I32)
                nc.sync.dma_start(idx_sb[:], x_idx[:])
                emb_tm = embp.tile([P, RC, C], F32)
                for rc in range(RC):
                    nc.gpsimd.indirect_dma_start(
                        out=emb_tm[:, rc, :],
                        out_offset=None,
                        in_=tok_emb[:],
                        in_offset=bass.IndirectOffsetOnAxis(
                            ap=idx_sb[:, rc : rc + 1], axis=0
                        ),
                    )
                pos_sb = embp.tile([P, RC, C], F32)
                nc.sync.dma_start(
                    pos_sb[:], pos_pc.rearrange("(c p) d -> p c d", p=P)
                )
                nc.vector.tensor_tensor(
                    emb_tm[:], emb_tm[:], pos_sb[:], ALU.add
                )
                for rc in range(RC):
                    for s in range(CS):
                        tp = ps.tile([P, P], F32, tag="mm")
                        nc.tensor.transpose(
                            tp[:], emb_tm[:, rc, s * P : (s + 1) * P], ident[:]
                        )
                        nc.vector.tensor_copy(
                            hT[:, s, rc * P : (rc + 1) * P], tp[:]
                        )

            # ---- LayerNorm helper (feature-major, stats via ones-matmul) ----
            def layer_norm(src, dst, sc_ap, bi_ap):
                # src [P, CS, RO] f32r -> dst [P, CS, RO] f32r normalized
                sq = statp.tile([P, CS, RO], F32R, tag="sq")
                nc.scalar.activation(sq[:], src[:], AF.Square)
                pm = ps.tile([P, RO], F32, tag="mm")
                pq = ps.tile([P, RO], F32, tag="mm")
                for s in range(CS):
                    nc.tensor.matmul(
                        pm[:], lhsT=ones_sb[:], rhs=src[:, s, :],
                        start=(s == 0), stop=(s == CS - 1),
                    )
                for s in range(CS):
                    nc.tensor.matmul(
                        pq[:], lhsT=ones_sb[:], rhs=sq[:, s, :],
                        start=(s == 0), stop=(s == CS - 1),
                    )
                m_sb = statp.tile([P, RO], F32, tag="m")
                nc.vector.tensor_scalar_mul(m_sb[:], pm[:], 1.0 / C)
                ms = statp.tile([P, RO], F32, tag="ms")
                nc.vector.tensor_mul(ms[:], m_sb[:], m_sb[:])
                var = statp.tile([P, RO], F32, tag="var")
                nc.vector.scalar_tensor_tensor(
                    out=var[:], in0=pq[:], scalar=1.0 / C, in1=ms[:],
                    op0=ALU.mult, op1=ALU.subtract,
                )
                std = statp.tile([P, RO], F32, tag="std")
                nc.scalar.activation(std[:], var[:], AF.Sqrt, bias=eps_t[:])
                rstd = statp.tile([P, RO], F32, tag="rstd")
                nc.vector.reciprocal(rstd[:], std[:])
                cen = statp.tile([P, CS, RO], F32, tag="sq", name="cen")
                nc.vector.tensor_tensor(
                    cen[:], src[:],
                    m_sb[:, None, :].to_broadcast((P, CS, RO)), ALU.subtract,
                )
                nc.vector.tensor_tensor(
                    dst[:], cen[:],
                    rstd[:, None, :].to_broadcast((P, CS, RO)), ALU.mult,
                )
                for s in range(CS):
                    nc.vector.tensor_scalar(
                        out=dst[:, s, :], in0=dst[:, s, :],
                        scalar1=sc_ap[s], scalar2=bi_ap[s],
                        op0=ALU.mult, op1=ALU.add,
                    )

            a_in = pers.tile([P, CS, RO], F32R)
            f_in = pers.tile([P, CS, RO], F32R)

            # ---- transformer layers ----
            for l in range(L):
                layer_norm(
                    hT, a_in,
                    [ln_sb[:, l, 0, 0, s : s + 1] for s in range(CS)],
                    [ln_sb[:, l, 0, 1, s : s + 1] for s in range(CS)],
                )

                # a_in -> DRAM -> AllGather
                ag1_in = dram.tile([CS, P, RO], F32R, tag=f"ag1i_{l}")
                nc.sync.dma_start(
                    ag1_in[:].rearrange("s p t -> p s t"), a_in[:]
                )
                ag1_out = dram.tile(
                    [R, CS, P, RO], F32R, tag=f"ag1o_{l}",
                    **({} if _nocoll else {"addr_space": "Shared"}),
                )
                coll("AllGather", ag1_in[:], ag1_out[:])

                with (
                    tc.tile_pool(name=f"attn_{l}", bufs=1) as ap_,
                    tc.tile_pool(name=f"attn2_{l}", bufs=4) as ap2,
                    tc.tile_pool(name=f"ps_at_{l}", bufs=2, space="PSUM") as ps_at,
                    tc.tile_pool(name=f"ps_av_{l}", bufs=2, space="PSUM") as ps_av,
                ):

                    wqk_sb = wsm.tile([P, CS, P], F32R, tag="wqk")
                    nc.sync.dma_start(
                        wqk_sb[:], wqk_pc[l].rearrange("s p m -> p s m")
                    )
                    wv_sb = wsm.tile([P, CS, HD], F32R, tag="wv")
                    nc.sync.dma_start(
                        wv_sb[:], wv_pc[l].rearrange("s p m -> p s m")
                    )
                    wpj_sb = wsm.tile([P, CS, C], F32R, tag="wpj")
                    nc.sync.dma_start(
                        wpj_sb[:], wpj_pc[l].rearrange("s p m -> p s m")
                    )

                    # q,k feature-major [128 = 64 dims x 2 batches, T]
                    qT = ap_.tile([P, T], F32R)
                    kT = ap_.tile([P, T], F32R)
                    v_sb = ap_.tile([P, NR // P, P], F32R)
                    nc.vector.tensor_copy(
                        v_sb[:, :, HD:P],
                        ones_sb[:, None, 0:HD].to_broadcast((P, NR // P, HD)),
                    )
                    for half in range(2):  # half == batch (rows half*2048..)
                        a_half = ap_.tile(
                            [P, CS, NR // 2], F32R, tag="a_half",
                            name=f"ah_{l}_{half}",
                        )
                        for b in range(RC):
                            nc.sync.dma_start(
                                a_half[:, :, b * RO : (b + 1) * RO],
                                ag1_out[RC * half + b].rearrange("s p t -> p s t"),
                            )
                        for n4 in range(RC):
                            pqk = ps.tile([P, 512], F32, tag="mm", name=f"pqk{half}_{n4}")
                            for s in range(CS):
                                nc.tensor.matmul(
                                    pqk[:], lhsT=wqk_sb[:, s, :],
                                    rhs=a_half[:, s, n4 * 512 : (n4 + 1) * 512],
                                    start=(s == 0), stop=(s == CS - 1),
                                )
                            col = n4 * 512
                            nc.vector.tensor_copy(
                                qT[64 * half : 64 * half + 64, col : col + 512],
                                pqk[0:64, :],
                            )
                            nc.vector.tensor_copy(
                                kT[64 * half : 64 * half + 64, col : col + 512],
                                pqk[64:128, :],
                            )
                        for ch in range(T // P):
                            pv = ps.tile([P, HD], F32, tag="mm", name=f"pv{half}_{ch}")
                            for s in range(CS):
                                nc.tensor.matmul(
                                    pv[:], lhsT=a_half[:, s, ch * P : (ch + 1) * P],
                                    rhs=wv_sb[:, s, :],
                                    start=(s == 0), stop=(s == CS - 1),
                                )
                            nc.vector.tensor_copy(
                                v_sb[:, half * (T // P) + ch, 0:HD], pv[:]
                            )

                    # causal attention, q blocks of 256, both batches in parallel
                    attn_sb = ap_.tile([P, T], F32R)
                    for qb in range(T // 256):
                        pav = [
                            ps_av.tile([P, 256], F32, tag="pav", name=f"pav{qb}_{b}")
                            for b in range(B)
                        ]
                        nkv = 2 * qb + 2
                        for j in range(nkv):
                            psc = [
                                ps_at.tile([P, 256], F32, tag="psc", name=f"psc{qb}_{j}_{b}")
                                for b in range(B)
                            ]
                            for b in range(B):
                                nc.tensor.matmul(
                                    psc[b][:],
                                    lhsT=kT[64 * b : 64 * b + 64, j * P : (j + 1) * P],
                                    rhs=qT[64 * b : 64 * b + 64, qb * 256 : qb * 256 + 256],
                                    start=True, stop=True,
                                    tile_position=(64 * b, 0),
                                )
                            for b in range(B):
                                ex = ap2.tile([P, 256], F32R, tag=f"ex{b}")
                                nc.scalar.activation(
                                    ex[:], psc[b][:], AF.Exp, scale=SCALE
                                )
                                if j >= nkv - 2:
                                    nc.vector.tensor_tensor(
                                        ex[:], ex[:],
                                        mask_sb[:, j - (nkv - 2), :], ALU.mult,
                                    )
                                nc.tensor.matmul(
                                    pav[b][:],
                                    lhsT=v_sb[:, b * (T // P) + j, :],
                                    rhs=ex[:],
                                    start=(j == 0), stop=(j == nkv - 1),
                                )
                        for b in range(B):
                            rec = ap2.tile([HD, 256], F32, tag=f"rec{b}")
                            nc.vector.reciprocal(rec[:], pav[b][64:128, :])
                            nc.vector.tensor_tensor(
                                attn_sb[64 * b : 64 * b + 64, qb * 256 : qb * 256 + 256],
                                pav[b][0:64, :], rec[:], ALU.mult,
                            )

                    # AllToAll: head blocks -> row blocks
                    a2a_in = dram.tile([R, HD, RO], F32R, tag=f"a2i_{l}")
                    for d in range(R):
                        nc.sync.dma_start(
                            a2a_in[d],
                            attn_sb[
                                64 * (d // RC) : 64 * (d // RC) + 64,
                                (d % RC) * RO : (d % RC + 1) * RO,
                            ],
                        )
                    a2a_out = dram.tile([R, HD, RO], F32R, tag=f"a2o_{l}")
                    coll("AllToAll", a2a_in[:], a2a_out[:])

                    # proj + residual (own rows)
                    af = ap_.tile([P, CS, RO], F32R)
                    for cc in range(CS):
                        nc.sync.dma_start(
                            af[:, cc, :],
                            a2a_out[2 * cc : 2 * cc + 2].rearrange(
                                "h d t -> (h d) t"
                            ),
                        )
                    for cc in range(CS):
                        ppr = ps.tile([P, RO], F32, tag="mm")
                        for s in range(CS):
                            nc.tensor.matmul(
                                ppr[:], lhsT=wpj_sb[:, s, cc * P : (cc + 1) * P],
                                rhs=af[:, s, :],
                                start=(s == 0), stop=(s == CS - 1),
                            )
                        nc.vector.scalar_tensor_tensor(
                            out=hT[:, cc, :], in0=ppr[:],
                            scalar=bpj_sb[:, l, cc : cc + 1], in1=hT[:, cc, :],
                            op0=ALU.add, op1=ALU.add,
                        )

                layer_norm(
                    hT, f_in,
                    [ln_sb[:, l, 1, 0, s : s + 1] for s in range(CS)],
                    [ln_sb[:, l, 1, 1, s : s + 1] for s in range(CS)],
                )

                # feed-forward
                with tc.tile_pool(name=f"ff_{l}", bufs=1) as ffp:
                    wf1_sb = ffp.tile([P, CS, FF], F32R)
                    nc.sync.dma_start(
                        wf1_sb[:], wf1_pc[l].rearrange("s p m -> p s m")
                    )
                    wf2_sb = ffp.tile([P, FS, C], F32R)
                    nc.sync.dma_start(
                        wf2_sb[:], wf2_pc[l].rearrange("s p m -> p s m")
                    )
                    hid_all = ffp.tile([P, FS, RO], F32R)
                    for hc in range(FS):
                        ph = ps.tile([P, RO], F32, tag="mm")
                        for s in range(CS):
                            nc.tensor.matmul(
                                ph[:], lhsT=wf1_sb[:, s, hc * P : (hc + 1) * P],
                                rhs=f_in[:, s, :],
                                start=(s == 0), stop=(s == CS - 1),
                            )
                        nc.scalar.activation(
                            hid_all[:, hc, :], ph[:], AF.Relu,
                            bias=bf1_sb[:, l, hc : hc + 1],
                        )
                    for cc in range(CS):
                        po = ps.tile([P, RO], F32, tag="mm")
                        for hc in range(FS):
                            nc.tensor.matmul(
                                po[:], lhsT=wf2_sb[:, hc, cc * P : (cc + 1) * P],
                                rhs=hid_all[:, hc, :],
                                start=(hc == 0), stop=(hc == FS - 1),
                            )
                        nc.vector.scalar_tensor_tensor(
                            out=hT[:, cc, :], in0=po[:],
                            scalar=bf2_sb[:, l, cc : cc + 1], in1=hT[:, cc, :],
                            op0=ALU.add, op1=ALU.add,
                        )

            # ---- final LN + AllGather ----
            layer_norm(
                hT, a_in,
                [lnf_sb[:, 0, s : s + 1] for s in range(CS)],
                [lnf_sb[:, 1, s : s + 1] for s in range(CS)],
            )
            ag3_in = dram.tile([CS, P, RO], F32R, tag="ag3i")
            nc.sync.dma_start(ag3_in[:].rearrange("s p t -> p s t"), a_in[:])
            ag3_out = dram.tile(
                [R, CS, P, RO], F32R, tag="ag3o",
                **({} if _nocoll else {"addr_space": "Shared"}),
            )
            coll("AllGather", ag3_in[:], ag3_out[:])

            # ---- lm_head (vocab chunk) + sum-exp partials ----
            with (
                tc.tile_pool(name="lm", bufs=1) as lmp,
                tc.tile_pool(name="lmw", bufs=2) as lmwp,
                tc.tile_pool(name="lms", bufs=3) as lms,
            ):
                lnf_all = lmp.tile([P, CS, NR], F32R)
                for b in range(R):
                    nc.sync.dma_start(
                        lnf_all[:, :, b * RO : (b + 1) * RO],
                        ag3_out[b].rearrange("s p t -> p s t"),
                    )
                se_sb = lmp.tile([P, (NR // P) * VT], F32)
                for vt in range(VT):
                    lmw_sb = lmwp.tile([P, CS, 512], F32R, tag="lmw")
                    nc.sync.dma_start(
                        lmw_sb[:],
                        lmw_pc[:, :, vt * 512 : (vt + 1) * 512].rearrange(
                            "s p m -> p s m"
                        ),
                    )
                    for rc in range(NR // P):
                        pl = ps.tile([P, 512], F32, tag="mm")
                        for s in range(CS):
                            nc.tensor.matmul(
                                pl[:], lhsT=lnf_all[:, s, rc * P : (rc + 1) * P],
                                rhs=lmw_sb[:, s, :],
                                start=(s == 0), stop=(s == CS - 1),
                            )
                        lg = lms.tile([P, 512], F32, tag="lg")
                        nc.vector.tensor_copy(lg[:], pl[:])
                        nc.sync.dma_start(
                            logits_o[rc * P : (rc + 1) * P, vt * 512 : (vt + 1) * 512],
                            lg[:],
                        )
                        scr = lms.tile([P, 512], F32, tag="scr")
                        nc.scalar.activation(
                            scr[:], lg[:], AF.Exp,
                            accum_out=se_sb[:, rc * VT + vt : rc * VT + vt + 1],
                        )
                se_tot = lmp.tile([P, NR // P], F32)
                nc.vector.reduce_sum(
                    se_tot[:],
                    se_sb[:].rearrange("p (rc v) -> p rc v", v=VT),
                    axis=mybir.AxisListType.X,
                )
                nc.sync.dma_start(sumexp_o[:], se_tot[:])

    nc.compile()
    return nc


_NC_CACHE = None
LAST_RESULT = None


def _get_nc():
    global _NC_CACHE
    if _NC_CACHE is None:
        _NC_CACHE = build_nc()
    return _NC_CACHE


def prepare_in_maps(x, target, tok_emb, pos_emb, ln1_scale, ln1_bias, wq, wk,
                    wv, w_proj, b_proj, ln2_scale, ln2_bias, w_ff1, b_ff1,
                    w_ff2, b_ff2, lnf_scale, lnf_bias, lm_w, lm_b):
    f32 = np.float32
    x = np.asarray(x, np.int32)
    target = np.asarray(target, np.int32)
    tok_emb = np.ascontiguousarray(np.asarray(tok_emb, f32))
    pos_emb = np.asarray(pos_emb, f32)
    ln_params = np.stack(
        [
            np.stack([np.asarray(ln1_scale, f32), np.asarray(ln1_bias, f32)], 1),
            np.stack([np.asarray(ln2_scale, f32), np.asarray(ln2_bias, f32)], 1),
        ],
        1,
    )  # [L, 2, 2, C]
    wq = np.asarray(wq, f32)
    wk = np.asarray(wk, f32)
    wv = np.asarray(wv, f32)
    w_proj = np.asarray(w_proj, f32)
    b_proj = np.asarray(b_proj, f32)
    w_ff1 = np.asarray(w_ff1, f32)
    b_ff1 = np.asarray(b_ff1, f32)
    w_ff2 = np.asarray(w_ff2, f32)
    b_ff2 = np.asarray(b_ff2, f32)
    lnf = np.stack([np.asarray(lnf_scale, f32), np.asarray(lnf_bias, f32)], 0)
    lm_w = np.asarray(lm_w, f32)
    lm_b = np.asarray(lm_b, f32)
    assert np.all(lm_b == 0.0) and np.all(b_proj == 0.0), \
        "device kernel folds zero biases; nonzero lm_b/b_proj unsupported"

    x_flat = x.reshape(-1)

    # vocab chunking: core r covers [r*6283, min(V, (r+1)*6283)) padded to VC
    CW = 6283
    widths = [min(V, (r + 1) * CW) - r * CW for r in range(R)]

    def perm_cpart(a):
        # [..., C] with C = s*128+p -> [..., CS, P] -> [P, ..., CS]
        sh = a.shape[:-1]
        b = a.reshape(*sh, CS, P)
        return np.ascontiguousarray(np.moveaxis(b, -1, 0))

    ln_pc = perm_cpart(ln_params)                     # [P, L, 2, 2, CS]
    lnf_pc = perm_cpart(lnf)                          # [P, 2, CS]
    bpj_pc = perm_cpart(b_proj)                       # [P, L, CS]
    bf2_pc = perm_cpart(b_ff2)                        # [P, L, CS]
    bf1_pc = np.ascontiguousarray(
        np.moveaxis(b_ff1.reshape(L, FS, P), -1, 0)
    )                                                 # [P, L, FS]

    ones_np = np.ones((P, P), f32)
    kvl = np.arange(P)[:, None]
    ql = np.arange(256)[None, :]
    masks_np = np.stack(
        [(ql >= kvl).astype(f32), (ql >= kvl + 128).astype(f32)], 0
    )

    wpj_np = w_proj.reshape(L, CS, P, C)
    wf1_np = w_ff1.reshape(L, CS, P, FF)
    wf2_np = w_ff2.reshape(L, FS, P, C)

    in_maps = []
    for r in range(R):
        rows = slice(r * RO, (r + 1) * RO)
        x_own = x_flat[rows].reshape(RC, P).T          # [P, RC]
        t0 = (r % RC) * RO
        pos_own = pos_emb[t0 : t0 + RO]
        wqk = np.concatenate([wq[:, r], wk[:, r]], axis=2)   # [L, C, 128]
        lmw = np.zeros((C, VC), f32)
        lmw[:, : widths[r]] = lm_w[:, r * CW : r * CW + widths[r]]
        in_maps.append(
            {
                "x_idx": np.ascontiguousarray(x_own),
                "pos_pc": np.ascontiguousarray(pos_own),
                "tok_emb": tok_emb,
                "ones_r": ones_np,
                "masks": masks_np,
                "ln_pc": ln_pc,
                "lnf_pc": lnf_pc,
                "bpj_pc": bpj_pc,
                "bf1_pc": bf1_pc,
                "bf2_pc": bf2_pc,
                "wqk_pc": np.ascontiguousarray(wqk.reshape(L, CS, P, P)),
                "wv_pc": np.ascontiguousarray(wv[:, r].reshape(L, CS, P, HD)),
                "wpj_pc": np.ascontiguousarray(wpj_np),
                "wf1_pc": np.ascontiguousarray(wf1_np),
                "wf2_pc": np.ascontiguousarray(wf2_np),
                "lmw_pc": np.ascontiguousarray(lmw.reshape(CS, P, VC)),
            }
        )

    return in_maps, widths


def kernel(**inputs):
    target = np.asarray(inputs["target"], np.int32)
    in_maps, widths = prepare_in_maps(**inputs)
    nc = _get_nc()
    import os as _os
    _tr = bool(_os.environ.get("KERNEL_TRACE"))
    res = run_bass_kernel_spmd(
        nc, in_maps, core_ids=list(range(R)),
        trace=_tr, trace_cores=[0] if _tr else None,
    )
    global LAST_RESULT
    LAST_RESULT = res
    outs = res.results

    logits = np.concatenate(
        [outs[r]["logits_o"][:, : widths[r]] for r in range(R)], axis=1
    )

    # CE loss from device partials (host glue on 4096-vectors)
    npad = np.array([VC - widths[r] for r in range(R)], f32)
    se = np.stack(
        [outs[r]["sumexp_o"].T.reshape(-1) - npad[r] for r in range(R)], 0
    )  # [R, NR]
    logz = np.log(se.sum(0))
    tgt = logits[np.arange(NR), target.reshape(-1)]
    loss = np.float32(-(tgt - logz).mean())

    return logits, loss
